# revision 15
# baseline (speedup 1.0000x reference)
"""LocallyConnectedXYZLayer Trainium2 kernel.

out[n,c,h,w] = sum_{dy,dx in 5x5} exp(-|xyz(n,:,h+dy-2,w+dx-2)-xyz(n,:,h,w)|^2/2)
               * (softmax*mask)(n,c,h+dy-2,w+dx-2)        (zero-padded)

Sharding: 8 cores = (batch n = core//2) x (W half = core%2).
Per-core layout: partitions = 2 w-chunks x 64 h rows; free dims = (dy, c, w).
The 5 dy window shifts are baked into host-prepared per-partition rows; dx
shifts are free-dim slices (an e-parity duplicate of the softmax slab keeps
every slice 4B-aligned so DVE ops run in the 2x dual-pump mode).

Engine split per (step s, dx) tau:
  Pool : diff = xyz_slab[dx] - xyz_center  (TT sub);  d2a = sq0 + sq1
  ACT  : sq = Square(diff);  g5 = Exp(-d2/2);  per step: PSUM->SBUF evac
  DVE  : d2 = d2a + sq2;  P[dy,c,w] = g5[dy,w] (bcast c) * sm_slab[e,dy,c,w]
  PE   : PSUM[c,w] += I128 @ P[dy]  for the 25 (dy,dx) planes of a step
         (identity matmuls, fp32 PSUM accumulation, 5 banks single-parity;
         the step s+1 start=True matmuls wait on the step-s evacuation)
Output is written bf16 and cast to f32 on the host.
"""

import sys
from contextlib import ExitStack

import numpy as np

sys.path.insert(0, "/opt/trn_rl_repo")

import ml_dtypes  # noqa: E402

import concourse.bass as bass  # noqa: E402
from concourse import mybir  # noqa: E402
from concourse.bass_utils import run_bass_kernel_spmd  # noqa: E402

BF16 = ml_dtypes.bfloat16

N, C, H, W = 4, 20, 64, 2048
KH = KW = 5
PAD = 2
HH = H + 2 * PAD  # 68 padded rows
WCORE = W // 2  # 1024 interior w per core
NSTEP = 4  # device steps
WS = WCORE // (2 * NSTEP)  # 128 interior w per (step, chunk)
WX = WS + 2 * PAD  # 132 w extent (halo 2 each side)
NTAU = NSTEP * KW  # 20
CW = C * WS  # 2560 psum accumulator columns (5 fp32 banks)
MM_CHUNKS = [(0, 4), (4, 4), (8, 4), (12, 4), (16, 4)]  # c-rows: 512 cols each
MM_PLANES = (0, 2, 3, 4)  # dy pair 0+1 is pre-folded on DVE
MM_PER_TAU = len(MM_PLANES) * len(MM_CHUNKS)  # 20
MM_PER_STEP = KW * MM_PER_TAU  # 100

_CACHE = {}


def _build_nc():
    """Raw-Bass program (no Tile): this toolchain's walrus codegen allows at
    most one sync-wait command per instruction, so all cross-engine sync is
    standalone wait_ge instructions plus one then_inc on producer ops."""
    nc = bass.Bass("TRN2", target_bir_lowering=False, debug=False)
    bf = mybir.dt.bfloat16
    f32 = mybir.dt.float32
    sm_d = nc.dram_tensor("sm_in", [NSTEP, 128, 2, KH, C, WX], bf,
                          kind="ExternalInput")
    xyz_d = nc.dram_tensor("xyz_in", [NSTEP, 128, KH, 3, WX], bf,
                           kind="ExternalInput")
    id_d = nc.dram_tensor("id_in", [128, 128], bf, kind="ExternalInput")
    out_d = nc.dram_tensor("out_d", [NSTEP, 128, C, WS], bf,
                           kind="ExternalOutput")

    def sb(name, shape, dt):
        return nc.alloc_sbuf_tensor(name, list(shape), dt).ap()

    sm_t = [sb(f"sm{i}", [128, 2, KH, C, WX], bf) for i in range(2)]
    xyz_t = [sb(f"xyz{i}", [128, KH, 3, WX], bf) for i in range(2)]
    id_t = sb("ident", [128, 128], bf)
    diff_t = [sb(f"diff{i}", [128, KH, 3, WS], bf) for i in range(2)]
    sq_t = [sb(f"sq{i}", [128, KH, 3, WS], bf) for i in range(2)]
    d2a_t = [sb(f"d2a{i}", [128, KH, WS], bf) for i in range(2)]
    d2_t = [sb(f"d2_{i}", [128, KH, WS], bf) for i in range(2)]
    g5_t = [sb(f"g5_{i}", [128, KH, WS], bf) for i in range(2)]
    p_t = [sb(f"p{i}", [128, KH, C, WS], bf) for i in range(2)]
    ob_t = [sb(f"ob{i}", [128, C, WS], bf) for i in range(2)]
    ps = nc.alloc_psum_tensor("ps", [128, CW], f32).ap()

    ADD, MULT, SUB = (mybir.AluOpType.add, mybir.AluOpType.mult,
                      mybir.AluOpType.subtract)
    AF = mybir.ActivationFunctionType

    with ExitStack() as ctx:
        load_sem = ctx.enter_context(nc.semaphore("load_sem"))  # xyz loads
        sme_sem = ctx.enter_context(nc.semaphore("sme_sem"))  # sm e=0 loads
        smo_sem = ctx.enter_context(nc.semaphore("smo_sem"))  # sm e=1 loads
        id_sem = ctx.enter_context(nc.semaphore("id_sem"))  # identity load
        store_sem = ctx.enter_context(nc.semaphore("store_sem"))
        sub_sem = ctx.enter_context(nc.semaphore("sub_sem"))  # pool sub done
        p1_sem = ctx.enter_context(nc.semaphore("p1_sem"))  # pool add1 done
        sq_sem = ctx.enter_context(nc.semaphore("sq_sem"))  # act square done
        a2_sem = ctx.enter_context(nc.semaphore("a2_sem"))  # dve add2 done
        exp_sem = ctx.enter_context(nc.semaphore("exp_sem"))  # act exp done
        mul_sem = ctx.enter_context(nc.semaphore("mul_sem"))  # dve mul done
        pe_sem = ctx.enter_context(nc.semaphore("pe_sem"))  # pe matmuls done
        evac_sem = ctx.enter_context(nc.semaphore("evac_sem"))  # act evac done
        block = ctx.enter_context(nc.Block())

        @block.sync
        def _(sync):
            sync.dma_start(id_t[:], id_d[:]).then_inc(id_sem, 16)
            for s in range(NSTEP):
                b = s % 2
                if s >= 2:
                    # input buffer reuse: step s-2 consumers must be done
                    sync.wait_ge(sub_sem, KW * (s - 1))
                    sync.wait_ge(mul_sem, KW * (s - 1))
                if s >= 1:
                    # DMA completions across steps are unordered; gate this
                    # step's loads on the previous step's completions so the
                    # cumulative thresholds imply the right data landed.
                    sync.wait_ge(load_sem, 16 * s)
                    sync.wait_ge(sme_sem, 16 * s)
                    sync.wait_ge(smo_sem, 16 * s)
                sync.dma_start(xyz_t[b][:], xyz_d[s]).then_inc(load_sem, 16)
                sync.dma_start(sm_t[b][:, 0], sm_d[s, :, 0]).then_inc(
                    sme_sem, 16)
                sync.dma_start(sm_t[b][:, 1], sm_d[s, :, 1]).then_inc(
                    smo_sem, 16)
                if s >= 1:
                    # store step s-1 once its evacuation is done
                    sync.wait_ge(evac_sem, s)
                    if s >= 2:
                        sync.wait_ge(store_sem, 16 * (s - 1))
                    sync.dma_start(out_d[s - 1],
                                   ob_t[(s - 1) % 2][:]).then_inc(
                                       store_sem, 16)
            sync.wait_ge(evac_sem, NSTEP)
            sync.wait_ge(store_sem, 16 * (NSTEP - 1))
            sync.dma_start(out_d[NSTEP - 1],
                           ob_t[(NSTEP - 1) % 2][:]).then_inc(store_sem, 16)

        @block.gpsimd
        def _(gpsimd):
            # Pool ops hold the DVE/GpSimd shared SBUF port pair; the pacing
            # gates below keep them inside the DVE mega-mul windows (whose
            # broadcast in0 leaves the shared pair free).  Emission order:
            # sub[0], sub[1], add1[0], sub[2], add1[1], ..., add1[NTAU-1].
            def sub(k):
                s, dxp = divmod(k, KW)
                b, t2 = s % 2, k % 2
                if dxp == 0:
                    gpsimd.wait_ge(load_sem, 16 * (s + 1))
                if k >= 2:
                    # diff buffer reuse (sq[k-2] done) + pacing: add2[k-2]
                    # fires as mul[k-3] issues, pinning sub[k] to a window
                    gpsimd.wait_ge(a2_sem, k - 1)
                xyz_c = xyz_t[b][:, 2, :, PAD:PAD + WS].unsqueeze(
                    1).broadcast_to([128, KH, 3, WS])
                gpsimd.tensor_tensor(
                    out=diff_t[t2][:], in0=xyz_t[b][:, :, :, dxp:dxp + WS],
                    in1=xyz_c, op=SUB).then_inc(sub_sem)

            def add1(k):
                t2 = k % 2
                gpsimd.wait_ge(sq_sem, k + 1)
                if k >= 2:
                    # d2a buffer reuse: DVE add2 of k-2 must be done
                    gpsimd.wait_ge(a2_sem, k - 1)
                gpsimd.tensor_tensor(
                    out=d2a_t[t2][:], in0=sq_t[t2][:, :, 0, :],
                    in1=sq_t[t2][:, :, 1, :], op=ADD).then_inc(p1_sem)

            sub(0)
            for k in range(1, NTAU):
                sub(k)
                add1(k - 1)
            add1(NTAU - 1)

        @block.scalar
        def _(scalar):
            def sq(k):
                t2 = k % 2
                scalar.wait_ge(sub_sem, k + 1)
                if k >= 2:
                    # sq buffer reuse: DVE add2 of k-2 must be done
                    scalar.wait_ge(a2_sem, k - 1)
                scalar.activation(out=sq_t[t2][:], in_=diff_t[t2][:],
                                  func=AF.Square).then_inc(sq_sem)

            def exp(k):
                t2 = k % 2
                scalar.wait_ge(a2_sem, k + 1)
                if k >= 2:
                    # g5 buffer reuse: DVE of tau k-2 (incl mul) must be done
                    scalar.wait_ge(mul_sem, k - 1)
                scalar.activation(out=g5_t[t2][:], in_=d2_t[t2][:],
                                  func=AF.Exp, scale=-0.5).then_inc(exp_sem)

            def evac(sev):
                scalar.wait_ge(pe_sem, MM_PER_STEP * (sev + 1))
                if sev >= 2:
                    # ob buffer reuse: store of step sev-2 must be done
                    scalar.wait_ge(store_sem, 16 * (sev - 1))
                scalar.activation(out=ob_t[sev % 2][:], in_=ps[:, 0:CW],
                                  func=AF.Copy).then_inc(evac_sem)

            sq(0)
            sq(1)
            exp(0)
            for k in range(2, NTAU):
                sq(k)
                exp(k - 1)
                # evacuate psum of step s-1 right after exp[5s]: it overlaps
                # the DVE mul of tau 5s and precedes PE's step-s matmuls
                if (k - 1) % KW == 0:
                    evac((k - 1) // KW - 1)
            exp(NTAU - 1)
            evac(NSTEP - 1)

        @block.vector
        def _(vector):
            def add2(tau):
                # ---- d2 = d2a + sq2 ----
                t2 = tau % 2
                vector.wait_ge(p1_sem, tau + 1)
                if tau >= 2:
                    # d2 buffer reuse: ACT exp of tau-2 must be done
                    vector.wait_ge(exp_sem, tau - 1)
                vector.tensor_tensor(
                    out=d2_t[t2][:], in0=d2a_t[t2][:],
                    in1=sq_t[t2][:, :, 2, :], op=ADD).then_inc(a2_sem)

            add2(0)
            for tau in range(NTAU):
                s, dxp = divmod(tau, KW)
                b, t2 = s % 2, tau % 2
                e, off = dxp % 2, dxp - dxp % 2
                if tau + 1 < NTAU:
                    add2(tau + 1)
                vector.wait_ge(exp_sem, tau + 1)
                if dxp == 0:
                    vector.wait_ge(sme_sem, 16 * (s + 1))
                elif dxp == 1:
                    vector.wait_ge(smo_sem, 16 * (s + 1))
                if tau >= 2:
                    # p buffer reuse: PE matmuls of tau-2 must be done
                    vector.wait_ge(pe_sem, MM_PER_TAU * (tau - 1))
                g_b = g5_t[t2][:].unsqueeze(2).broadcast_to([128, KH, C, WS])
                vector.tensor_tensor(
                    out=p_t[t2][:], in0=g_b,
                    in1=sm_t[b][:, e, :, :, off:off + WS],
                    op=MULT)
                # pre-fold one dy pair so the (throttled) PE only accumulates
                # 4 planes.  This 2-stream add grabs the DVE/GpSimd shared
                # SBUF port pair, so it must never overlap a Pool op: wait
                # for this window's Pool pair (sub[tau+3], add1[tau+2]) to
                # finish; Pool's next sub is in turn gated on add2[tau+2],
                # which DVE emits after this fold -- a self-synchronizing
                # lockstep that keeps Pool strictly inside the mul windows.
                vector.wait_ge(p1_sem, min(tau + 3, NTAU))
                vector.tensor_tensor(
                    out=p_t[t2][:, 0], in0=p_t[t2][:, 0], in1=p_t[t2][:, 1],
                    op=ADD).then_inc(mul_sem)

        @block.tensor
        def _(tensor):
            tensor.wait_ge(id_sem, 16)
            for tau in range(NTAU):
                s, dxp = divmod(tau, KW)
                t2 = tau % 2
                tensor.wait_ge(mul_sem, tau + 1)
                if dxp == 0 and s >= 1:
                    # single-parity psum: evacuation of step s-1 must be done
                    tensor.wait_ge(evac_sem, s)
                nmm = 0
                for dy in MM_PLANES:
                    for c0, cn in MM_CHUNKS:
                        nmm += 1
                        mm = tensor.matmul(
                            ps[:, c0 * WS:(c0 + cn) * WS],
                            id_t[:],
                            p_t[t2][:, dy, c0:c0 + cn, :],
                            start=(dxp == 0 and dy == 0),
                            stop=(dxp == KW - 1 and dy == KH - 1),
                            skip_group_check=True,
                        )
                        if nmm == MM_PER_TAU:
                            mm.then_inc(pe_sem, MM_PER_TAU)

    return nc


def _prep_core(xyz, softmax, mask, core):
    """Build the per-core dy-baked slab arrays (host side, bf16).

    Row layout: partition p (0..127) = chunk (p//64) x h row (p%64); the
    dy dim holds the 5 shifted window rows h+dy (in padded coords)."""
    n, half = core // 2, core % 2
    w0 = WCORE * half
    wp_sz = WCORE + 2 * PAD + 1
    lo, hi = w0 - PAD, w0 + WCORE + PAD + 1
    glo, ghi = max(lo, 0), min(hi, W)

    smm = (softmax[n][:, :, glo:ghi]
           * mask[n][None, :, glo:ghi].astype(np.float32))
    smp = np.zeros((HH, C, wp_sz), BF16)
    smp[PAD:PAD + H, :, glo - lo:ghi - lo] = smm.transpose(1, 0, 2).astype(
        BF16)
    xyzp = np.zeros((HH, 3, wp_sz), BF16)
    xyzp[PAD:PAD + H, :, glo - lo:ghi - lo] = (
        xyz[n][:, :, glo:ghi].transpose(1, 0, 2).astype(BF16))

    sm5 = np.empty((NSTEP, 128, 2, KH, C, WX), BF16)
    xyz5 = np.empty((NSTEP, 128, KH, 3, WX), BF16)
    for s in range(NSTEP):
        for chunk in range(2):
            wb = WS * s + (WCORE // 2) * chunk
            pr = slice(64 * chunk, 64 * chunk + 64)
            for dy in range(KH):
                for e in range(2):
                    sm5[s, pr, e, dy] = smp[dy:dy + 64, :, wb + e:wb + e + WX]
                xyz5[s, pr, dy] = xyzp[dy:dy + 64, :, wb:wb + WX]
    ident = np.eye(128, dtype=BF16)
    return {"sm_in": sm5, "xyz_in": xyz5, "id_in": ident}


def make_in_maps(xyz, softmax, mask):
    return [_prep_core(xyz, softmax, mask, k) for k in range(8)]


def assemble_out(results):
    out = np.empty((N, C, H, W), np.float32)
    for core in range(8):
        n, half = core // 2, core % 2
        w0 = WCORE * half
        o = np.asarray(results[core]["out_d"]).astype(np.float32)
        # [s, chunk*64+h, c, j] -> [c, h, WS*s + 512*chunk + j]
        o = o.reshape(NSTEP, 2, H, C, WS)
        # -> [c, h, chunk, s, j]
        out[n, :, :, w0:w0 + WCORE] = o.transpose(3, 2, 1, 0, 4).reshape(
            C, H, WCORE)
    return out


def get_nc():
    if "nc" not in _CACHE:
        _CACHE["nc"] = _build_nc()
    return _CACHE["nc"]


def kernel(xyz, softmax, mask, trace=False, trace_kwargs=None):
    nc = get_nc()
    in_maps = make_in_maps(np.asarray(xyz), np.asarray(softmax),
                           np.asarray(mask))
    res = run_bass_kernel_spmd(nc, in_maps, list(range(8)), trace=trace,
                               **(trace_kwargs or {}))
    out = assemble_out(res.results)
    if trace:
        return out, res
    return out


# revision 19
# speedup vs baseline: 1.3240x; 1.3240x over previous
"""LocallyConnectedXYZLayer Trainium2 kernel.

out[n,c,h,w] = sum_{dy,dx in 5x5} exp(-|xyz(n,:,h+dy-2,w+dx-2)-xyz(n,:,h,w)|^2/2)
               * (softmax*mask)(n,c,h+dy-2,w+dx-2)        (zero-padded)

Sharding: 8 cores = (batch n = core//2) x (W half = core%2).
Per-core layout: partitions = 2 w-chunks x 64 h rows; free dims = (dy, c, w).
The 5 dy window shifts are baked into host-prepared per-partition rows; dx
shifts are free-dim slices (an e-parity duplicate of the softmax slab keeps
every slice 4B-aligned so DVE ops run in the 2x dual-pump mode).

Engine split per (step s, dx) tau:
  Pool : diff = xyz_slab[dx] - xyz_center  (TT sub);  d2a = sq0 + sq1
  ACT  : sq = Square(diff);  g5 = Exp(-d2/2);  per step: PSUM->SBUF evac
  DVE  : d2 = d2a + sq2;  P[dy,c,w] = g5[dy,w] (bcast c) * sm_slab[e,dy,c,w]
  PE   : PSUM[c,w] += I128 @ P[dy]  for the 25 (dy,dx) planes of a step
         (identity matmuls, fp32 PSUM accumulation, 5 banks single-parity;
         the step s+1 start=True matmuls wait on the step-s evacuation)
Output is written bf16 and cast to f32 on the host.
"""

import sys
from contextlib import ExitStack

import numpy as np

sys.path.insert(0, "/opt/trn_rl_repo")

import ml_dtypes  # noqa: E402

import concourse.bass as bass  # noqa: E402
from concourse import mybir  # noqa: E402
from concourse.bass_utils import run_bass_kernel_spmd  # noqa: E402

BF16 = ml_dtypes.bfloat16

N, C, H, W = 4, 20, 64, 2048
KH = KW = 5
PAD = 2
HH = H + 2 * PAD  # 68 padded rows
WCORE = W // 2  # 1024 interior w per core
NSTEP = 4  # device steps
WS = WCORE // (2 * NSTEP)  # 128 interior w per (step, chunk)
WX = WS + 2 * PAD  # 132 w extent (halo 2 each side)
NTAU = NSTEP * KW  # 20
CW = C * WS  # 2560 psum accumulator columns (5 fp32 banks)
MM_CHUNKS = [(0, 4), (4, 4), (8, 4), (12, 4), (16, 4)]  # c-rows: 512 cols each
MM_PLANES = (0, 1, 2, 3, 4)
MM_PER_TAU = len(MM_PLANES) * len(MM_CHUNKS)  # 25
MM_PER_STEP = KW * MM_PER_TAU  # 125

_CACHE = {}


def _build_nc():
    """Raw-Bass program (no Tile): this toolchain's walrus codegen allows at
    most one sync-wait command per instruction, so all cross-engine sync is
    standalone wait_ge instructions plus one then_inc on producer ops."""
    nc = bass.Bass("TRN2", target_bir_lowering=False, debug=False)
    bf = mybir.dt.bfloat16
    f32 = mybir.dt.float32
    sm_d = nc.dram_tensor("sm_in", [NSTEP, 128, 2, KH, C, WX], bf,
                          kind="ExternalInput")
    xyz_d = nc.dram_tensor("xyz_in", [NSTEP, 128, KH, 3, WX], bf,
                           kind="ExternalInput")
    id_d = nc.dram_tensor("id_in", [128, 128], bf, kind="ExternalInput")
    out_d = nc.dram_tensor("out_d", [NSTEP, 128, C, WS], bf,
                           kind="ExternalOutput")

    def sb(name, shape, dt):
        return nc.alloc_sbuf_tensor(name, list(shape), dt).ap()

    sm_t = [sb(f"sm{i}", [128, 2, KH, C, WX], bf) for i in range(2)]
    xyz_t = [sb(f"xyz{i}", [128, KH, 3, WX], bf) for i in range(2)]
    id_t = sb("ident", [128, 128], bf)
    diff_t = [sb(f"diff{i}", [128, KH, 3, WS], bf) for i in range(2)]
    sq_t = [sb(f"sq{i}", [128, KH, 3, WS], bf) for i in range(2)]
    d2a_t = [sb(f"d2a{i}", [128, KH, WS], bf) for i in range(2)]
    d2_t = [sb(f"d2_{i}", [128, KH, WS], bf) for i in range(2)]
    g5_t = [sb(f"g5_{i}", [128, KH, WS], bf) for i in range(2)]
    p_t = [sb(f"p{i}", [128, KH, C, WS], bf) for i in range(2)]
    ob_t = [sb(f"ob{i}", [128, C, WS], bf) for i in range(2)]
    ps = nc.alloc_psum_tensor("ps", [128, CW], f32).ap()

    ADD, MULT, SUB = (mybir.AluOpType.add, mybir.AluOpType.mult,
                      mybir.AluOpType.subtract)
    AF = mybir.ActivationFunctionType

    with ExitStack() as ctx:
        load_sem = ctx.enter_context(nc.semaphore("load_sem"))  # xyz loads
        sme_sem = ctx.enter_context(nc.semaphore("sme_sem"))  # sm e=0 loads
        smo_sem = ctx.enter_context(nc.semaphore("smo_sem"))  # sm e=1 loads
        id_sem = ctx.enter_context(nc.semaphore("id_sem"))  # identity load
        store_sem = ctx.enter_context(nc.semaphore("store_sem"))
        sub_sem = ctx.enter_context(nc.semaphore("sub_sem"))  # pool sub done
        p1_sem = ctx.enter_context(nc.semaphore("p1_sem"))  # pool add1 done
        sq_sem = ctx.enter_context(nc.semaphore("sq_sem"))  # act square done
        a2_sem = ctx.enter_context(nc.semaphore("a2_sem"))  # dve add2 done
        exp_sem = ctx.enter_context(nc.semaphore("exp_sem"))  # act exp done
        mul_sem = ctx.enter_context(nc.semaphore("mul_sem"))  # dve mul done
        pe_sem = ctx.enter_context(nc.semaphore("pe_sem"))  # pe matmuls done
        evac_sem = ctx.enter_context(nc.semaphore("evac_sem"))  # act evac done
        block = ctx.enter_context(nc.Block())

        @block.sync
        def _(sync):
            sync.dma_start(id_t[:], id_d[:]).then_inc(id_sem, 16)
            for s in range(NSTEP):
                b = s % 2
                if s >= 2:
                    # input buffer reuse: step s-2 consumers must be done
                    sync.wait_ge(sub_sem, KW * (s - 1))
                    sync.wait_ge(mul_sem, KW * (s - 1))
                if s >= 1:
                    # DMA completions across steps are unordered; gate this
                    # step's loads on the previous step's completions so the
                    # cumulative thresholds imply the right data landed.
                    sync.wait_ge(load_sem, 16 * s)
                    sync.wait_ge(sme_sem, 16 * s)
                    sync.wait_ge(smo_sem, 16 * s)
                sync.dma_start(xyz_t[b][:], xyz_d[s]).then_inc(load_sem, 16)
                sync.dma_start(sm_t[b][:, 0], sm_d[s, :, 0]).then_inc(
                    sme_sem, 16)
                sync.dma_start(sm_t[b][:, 1], sm_d[s, :, 1]).then_inc(
                    smo_sem, 16)
                if s >= 1:
                    # store step s-1 once its evacuation is done
                    sync.wait_ge(evac_sem, s)
                    if s >= 2:
                        sync.wait_ge(store_sem, 16 * (s - 1))
                    sync.dma_start(out_d[s - 1],
                                   ob_t[(s - 1) % 2][:]).then_inc(
                                       store_sem, 16)
            sync.wait_ge(evac_sem, NSTEP)
            sync.wait_ge(store_sem, 16 * (NSTEP - 1))
            sync.dma_start(out_d[NSTEP - 1],
                           ob_t[(NSTEP - 1) % 2][:]).then_inc(store_sem, 16)

        # Pool/GpSimd is left idle: its ops lose the shared SBUF port to
        # both DVE 2-stream ops and PE matmul bursts (~2x stretch), and with
        # PE busy >80% of every window there is no clean slot for it.

        @block.scalar
        def _(scalar):
            def sq(k):
                t2 = k % 2
                scalar.wait_ge(sub_sem, k + 1)
                if k >= 2:
                    # sq buffer reuse: DVE add2 of k-2 must be done
                    scalar.wait_ge(a2_sem, k - 1)
                scalar.activation(out=sq_t[t2][:], in_=diff_t[t2][:],
                                  func=AF.Square).then_inc(sq_sem)

            def exp(k):
                t2 = k % 2
                scalar.wait_ge(a2_sem, k + 1)
                if k >= 2:
                    # g5 buffer reuse: DVE mul of tau k-2 must be done
                    scalar.wait_ge(mul_sem, k - 1)
                scalar.activation(out=g5_t[t2][:], in_=d2_t[t2][:],
                                  func=AF.Exp, scale=-0.5).then_inc(exp_sem)

            def evac(sev):
                scalar.wait_ge(pe_sem, MM_PER_STEP * (sev + 1))
                if sev >= 2:
                    # ob buffer reuse: store of step sev-2 must be done
                    scalar.wait_ge(store_sem, 16 * (sev - 1))
                scalar.activation(out=ob_t[sev % 2][:], in_=ps[:, 0:CW],
                                  func=AF.Copy).then_inc(evac_sem)

            sq(0)
            for k in range(NTAU):
                exp(k)
                # evacuate psum of step s-1 right after exp[5s]: it overlaps
                # the DVE sub/mul of tau 5s and precedes PE's step-s matmuls
                if k % KW == 0 and k >= KW:
                    evac(k // KW - 1)
                if k + 1 < NTAU:
                    sq(k + 1)
            evac(NSTEP - 1)

        @block.vector
        def _(vector):
            def sub(k):
                s, dxp = divmod(k, KW)
                b, t2 = s % 2, k % 2
                if dxp == 0:
                    vector.wait_ge(load_sem, 16 * (s + 1))
                if k >= 2:
                    # diff buffer reuse: ACT square of k-2 must be done
                    vector.wait_ge(sq_sem, k - 1)
                xyz_c = xyz_t[b][:, 2, :, PAD:PAD + WS].unsqueeze(
                    1).broadcast_to([128, KH, 3, WS])
                vector.tensor_tensor(
                    out=diff_t[t2][:], in0=xyz_t[b][:, :, :, dxp:dxp + WS],
                    in1=xyz_c, op=SUB).then_inc(sub_sem)

            def add1(k):
                t2 = k % 2
                vector.wait_ge(sq_sem, k + 1)
                vector.tensor_tensor(
                    out=d2a_t[t2][:], in0=sq_t[t2][:, :, 0, :],
                    in1=sq_t[t2][:, :, 1, :], op=ADD).then_inc(p1_sem)

            def add2(k):
                # same-engine RAW on d2a: SBUF writes are pipelined, so wait
                # for add1's commit before reading its output
                t2 = k % 2
                vector.wait_ge(p1_sem, k + 1)
                if k >= 2:
                    # d2 buffer reuse: ACT exp of k-2 must be done
                    vector.wait_ge(exp_sem, k - 1)
                vector.tensor_tensor(
                    out=d2_t[t2][:], in0=d2a_t[t2][:],
                    in1=sq_t[t2][:, :, 2, :], op=ADD).then_inc(a2_sem)

            sub(0)
            for tau in range(NTAU):
                s, dxp = divmod(tau, KW)
                b, t2 = s % 2, tau % 2
                e, off = dxp % 2, dxp - dxp % 2
                # d2 chain for this tau, then prefetch-sub for the next: the
                # ACT exp of this tau hides behind the sub
                add1(tau)
                add2(tau)
                if tau + 1 < NTAU:
                    sub(tau + 1)
                vector.wait_ge(exp_sem, tau + 1)
                if dxp == 0:
                    vector.wait_ge(sme_sem, 16 * (s + 1))
                elif dxp == 1:
                    vector.wait_ge(smo_sem, 16 * (s + 1))
                if tau >= 2:
                    # p buffer reuse: PE matmuls of tau-2 must be done
                    vector.wait_ge(pe_sem, MM_PER_TAU * (tau - 1))
                g_b = g5_t[t2][:].unsqueeze(2).broadcast_to([128, KH, C, WS])
                vector.tensor_tensor(
                    out=p_t[t2][:], in0=g_b,
                    in1=sm_t[b][:, e, :, :, off:off + WS],
                    op=MULT).then_inc(mul_sem)

        @block.tensor
        def _(tensor):
            tensor.wait_ge(id_sem, 16)
            for tau in range(NTAU):
                s, dxp = divmod(tau, KW)
                t2 = tau % 2
                tensor.wait_ge(mul_sem, tau + 1)
                if dxp == 0 and s >= 1:
                    # single-parity psum: evacuation of step s-1 must be done
                    tensor.wait_ge(evac_sem, s)
                nmm = 0
                for dy in MM_PLANES:
                    for c0, cn in MM_CHUNKS:
                        nmm += 1
                        mm = tensor.matmul(
                            ps[:, c0 * WS:(c0 + cn) * WS],
                            id_t[:],
                            p_t[t2][:, dy, c0:c0 + cn, :],
                            start=(dxp == 0 and dy == 0),
                            stop=(dxp == KW - 1 and dy == KH - 1),
                            skip_group_check=True,
                        )
                        if nmm == MM_PER_TAU:
                            mm.then_inc(pe_sem, MM_PER_TAU)

    return nc


def _prep_core(xyz, softmax, mask, core):
    """Build the per-core dy-baked slab arrays (host side, bf16).

    Row layout: partition p (0..127) = chunk (p//64) x h row (p%64); the
    dy dim holds the 5 shifted window rows h+dy (in padded coords)."""
    n, half = core // 2, core % 2
    w0 = WCORE * half
    wp_sz = WCORE + 2 * PAD + 1
    lo, hi = w0 - PAD, w0 + WCORE + PAD + 1
    glo, ghi = max(lo, 0), min(hi, W)

    smm = (softmax[n][:, :, glo:ghi]
           * mask[n][None, :, glo:ghi].astype(np.float32))
    smp = np.zeros((HH, C, wp_sz), BF16)
    smp[PAD:PAD + H, :, glo - lo:ghi - lo] = smm.transpose(1, 0, 2).astype(
        BF16)
    xyzp = np.zeros((HH, 3, wp_sz), BF16)
    xyzp[PAD:PAD + H, :, glo - lo:ghi - lo] = (
        xyz[n][:, :, glo:ghi].transpose(1, 0, 2).astype(BF16))

    sm5 = np.empty((NSTEP, 128, 2, KH, C, WX), BF16)
    xyz5 = np.empty((NSTEP, 128, KH, 3, WX), BF16)
    for s in range(NSTEP):
        for chunk in range(2):
            wb = WS * s + (WCORE // 2) * chunk
            pr = slice(64 * chunk, 64 * chunk + 64)
            for dy in range(KH):
                for e in range(2):
                    sm5[s, pr, e, dy] = smp[dy:dy + 64, :, wb + e:wb + e + WX]
                xyz5[s, pr, dy] = xyzp[dy:dy + 64, :, wb:wb + WX]
    ident = np.eye(128, dtype=BF16)
    return {"sm_in": sm5, "xyz_in": xyz5, "id_in": ident}


def make_in_maps(xyz, softmax, mask):
    return [_prep_core(xyz, softmax, mask, k) for k in range(8)]


def assemble_out(results):
    out = np.empty((N, C, H, W), np.float32)
    for core in range(8):
        n, half = core // 2, core % 2
        w0 = WCORE * half
        o = np.asarray(results[core]["out_d"]).astype(np.float32)
        # [s, chunk*64+h, c, j] -> [c, h, WS*s + 512*chunk + j]
        o = o.reshape(NSTEP, 2, H, C, WS)
        # -> [c, h, chunk, s, j]
        out[n, :, :, w0:w0 + WCORE] = o.transpose(3, 2, 1, 0, 4).reshape(
            C, H, WCORE)
    return out


def get_nc():
    if "nc" not in _CACHE:
        _CACHE["nc"] = _build_nc()
    return _CACHE["nc"]


def kernel(xyz, softmax, mask, trace=False, trace_kwargs=None):
    nc = get_nc()
    in_maps = make_in_maps(np.asarray(xyz), np.asarray(softmax),
                           np.asarray(mask))
    res = run_bass_kernel_spmd(nc, in_maps, list(range(8)), trace=trace,
                               **(trace_kwargs or {}))
    out = assemble_out(res.results)
    if trace:
        return out, res
    return out
